# revision 17
# baseline (speedup 1.0000x reference)
"""Trainium2 Bass kernel for NTTailAttentionModel2Softmax.

Model: embedding -> single-layer LSTM (S=1024, B=64, E=256, H=512) ->
tail attention (query = last batch element's final hidden) -> concat ->
vocab projection (V=10000) -> log_softmax.  Returns (logp, hT, cT).

Sharding: data-parallel over batch across 8 cores (8 batches/core, all
weights replicated).  The attention query needs batch 63's hidden
trajectory, so every core redundantly computes batch 63 as a 9th local
column -- zero collectives.

Per-core layout (all "T" tensors keep the reduction dim on partitions):
  - gather:  indirect-DMA emb rows -> PE-transpose -> xT [e, (b,t)] bf16
  - xproj:   xprojT[g, (b,t)] = w_ihT.T @ xT + bias -> DRAM [t,16,9,128] bf16
  - LSTM:    per step, 64 matmuls (16 gate-tiles x 4 h-chunks) with
             stationary w_hhT bf16 tiles, moving h [128,9];
             gates kept as [128 gate-part, 9 batch] so DVE/ACT run on all
             128 lanes; outs stored bf16 in SBUF as 4 x [128, 9*1024]
  - attn:    v = attn_w.T @ q (PE), scores via v-stationary matmuls,
             softmax on [8,1024], context via broadcast-mul-reduce
  - final:   logits[8,10000] = catT-stationary matmuls streaming h2o_wT
             bf16 from DRAM, then log_softmax on-device
"""

import sys

for p in ("/opt/trn_rl_repo",):
    if p not in sys.path:
        sys.path.insert(0, p)

import numpy as np
import ml_dtypes

SEQ, B, E, H, V = 1024, 64, 256, 512, 10000
NCORES = 8
BL = 9          # local batch columns: 8 own + replicated batch 63
G = 4 * H       # 2048 gate rows
MT = G // 128   # 16 gate tiles
KC = H // 128   # 4 h chunks
EC = E // 128   # 2 e chunks
HOW_NT = 20     # vocab tiles of 500
NV = V // HOW_NT

TRACE = False
LAST_RESULTS = None

_CACHE = {}

BF16 = ml_dtypes.bfloat16


def _build():
    import concourse.bacc as bacc
    import concourse.tile as tile
    from concourse import bass, mybir
    from concourse.masks import make_identity

    f32 = mybir.dt.float32
    bf16 = mybir.dt.bfloat16
    i32 = mybir.dt.int32
    AX = mybir.AxisListType
    OP = mybir.AluOpType
    AF = mybir.ActivationFunctionType

    nc = bacc.Bacc("TRN2", target_bir_lowering=False)

    # ---- DRAM I/O ----
    idx_d = nc.dram_tensor("idx", [BL * 8, 128, 1], i32, kind="ExternalInput")
    emb_d = nc.dram_tensor("emb", [V, E], f32, kind="ExternalInput")
    wih_d = nc.dram_tensor("wih", [EC, 128, G], bf16, kind="ExternalInput")
    whh_d = nc.dram_tensor("whh", [KC, 128, G], bf16, kind="ExternalInput")
    bias_d = nc.dram_tensor("bias", [128, MT], f32, kind="ExternalInput")
    h0_d = nc.dram_tensor("h0t", [128, KC * BL], bf16, kind="ExternalInput")
    c0_d = nc.dram_tensor("c0t", [128, KC * BL], f32, kind="ExternalInput")
    attnw_d = nc.dram_tensor("attnw", [KC, 128, H], f32, kind="ExternalInput")
    how_d = nc.dram_tensor("how", [8, 128, V], bf16, kind="ExternalInput")
    hob_d = nc.dram_tensor("hob", [8, V], f32, kind="ExternalInput")

    logp_o = nc.dram_tensor("logp_o", [8, V], f32, kind="ExternalOutput")
    # hT/cT leave the device in [p, k, b] layout; host reshapes to [8, H]
    hT_o = nc.dram_tensor("hT_o", [128, KC * 8], f32, kind="ExternalOutput")
    cT_o = nc.dram_tensor("cT_o", [128, KC * 8], f32, kind="ExternalOutput")

    # xprojT staged in DRAM: [m, p, b, t] bf16
    xpd = nc.dram_tensor("xpd", [MT, 128, BL, SEQ], bf16)
    # logits bounce buffer (SBUF is too narrow to hold [8, V] rows)
    logits_d = nc.dram_tensor("logits_d", [8, V], f32)

    # ---- persistent SBUF ----
    whh_sb = nc.alloc_sbuf_tensor("whh_sb", [128, KC * G], bf16)
    wih_sb = nc.alloc_sbuf_tensor("wih_sb", [128, EC * G], bf16)
    bias_sb = nc.alloc_sbuf_tensor("bias_sb", [128, MT], f32)
    h0_sb = nc.alloc_sbuf_tensor("h0_sb", [128, KC * BL], bf16)
    c_sb = nc.alloc_sbuf_tensor("c_sb", [128, KC * BL], f32)
    h_sb = nc.alloc_sbuf_tensor("h_sb", [128, KC * BL], f32)
    attnw_sb = nc.alloc_sbuf_tensor("attnw_sb", [128, KC * H], f32)
    outsT = [
        nc.alloc_sbuf_tensor(f"outsT{k}", [128, BL * SEQ], bf16) for k in range(KC)
    ]
    ident = nc.alloc_sbuf_tensor("ident", [128, 128], f32)
    scores_sb = nc.alloc_sbuf_tensor("scores_sb", [8, 1024], f32)
    wts_sb = nc.alloc_sbuf_tensor("wts_sb", [8, 1024], f32)
    wtsn_sb = nc.alloc_sbuf_tensor("wtsn_sb", [8, 1024], bf16)
    ones_sb = nc.alloc_sbuf_tensor("ones_sb", [1, 128], bf16)
    vbf_sb = nc.alloc_sbuf_tensor("vbf_sb", [128, KC], bf16)
    cntxT_sb = nc.alloc_sbuf_tensor("cntxT_sb", [128, KC * 8], f32)
    catT_sb = nc.alloc_sbuf_tensor("catT_sb", [128, 64], bf16)

    with tile.TileContext(nc) as tc:
        # ---------- load weights / init ----------
        nc.sync.dma_start(whh_sb[:].rearrange("p (k g) -> p k g", g=G), whh_d[:].rearrange("k p g -> p k g"))
        nc.sync.dma_start(wih_sb[:].rearrange("p (k g) -> p k g", g=G), wih_d[:].rearrange("k p g -> p k g"))
        nc.sync.dma_start(bias_sb[:], bias_d[:])
        nc.sync.dma_start(h0_sb[:], h0_d[:])
        nc.sync.dma_start(c_sb[:], c0_d[:])
        nc.sync.dma_start(attnw_sb[:].rearrange("p (k h) -> p k h", h=H), attnw_d[:].rearrange("k p h -> p k h"))
        make_identity(nc, ident[:])
        nc.gpsimd.memset(ones_sb[:], 1.0)

        # ---------- phase A: gather + xproj precompute ----------
        with (
            tc.tile_pool(name="ga", bufs=3) as ga,
            tc.tile_pool(name="gap", bufs=3, space="PSUM") as gap,
            tc.tile_pool(name="xap", bufs=4, space="PSUM") as xap,
        ):
            # 18 N-blocks of 512 cols; block j covers local batch bl=j//2,
            # t-half th=j%2; built from 4 gather tiles of 128 steps each.
            for j in range(2 * BL):
                bl, th = j // 2, j % 2
                xt_blk = [
                    ga.tile([128, 512], bf16, tag=f"xt{ec}", name=f"xt{ec}_{j}")
                    for ec in range(EC)
                ]
                for u in range(4):
                    i = bl * 8 + th * 4 + u
                    idx_sb = ga.tile([128, 1], i32, tag="idx")
                    nc.sync.dma_start(idx_sb[:], idx_d[i])
                    x_t = ga.tile([128, E], f32, tag="xg")
                    nc.gpsimd.indirect_dma_start(
                        out=x_t[:],
                        out_offset=None,
                        in_=emb_d[:],
                        in_offset=bass.IndirectOffsetOnAxis(ap=idx_sb[:, :1], axis=0),
                    )
                    for ec in range(EC):
                        pt = gap.tile([128, 128], f32, tag="tp")
                        nc.tensor.transpose(
                            out=pt[:], in_=x_t[:, ec * 128 : (ec + 1) * 128],
                            identity=ident[:],
                        )
                        nc.vector.tensor_copy(
                            out=xt_blk[ec][:, u * 128 : (u + 1) * 128], in_=pt[:]
                        )
                for m in range(MT):
                    px = xap.tile([128, 512], f32, tag="px")
                    for ec in range(EC):
                        nc.tensor.matmul(
                            out=px[:],
                            lhsT=wih_sb[:, ec * G + m * 128 : ec * G + (m + 1) * 128],
                            rhs=xt_blk[ec][:],
                            start=(ec == 0),
                            stop=(ec == EC - 1),
                        )
                    xo = ga.tile([128, 512], bf16, tag="xo")
                    # add per-gate bias (per-partition scalar) + cast to bf16
                    nc.vector.tensor_scalar_add(
                        out=xo[:], in0=px[:], scalar1=bias_sb[:, m : m + 1]
                    )
                    nc.sync.dma_start(
                        out=xpd[m, :, bl, th * 512 : (th + 1) * 512],
                        in_=xo[:],
                    )

        # ---------- phase B: LSTM recurrence ----------
        with (
            tc.tile_pool(name="xpp", bufs=2) as xpp,
            tc.tile_pool(name="lp", bufs=2) as lp,
            tc.tile_pool(name="gp", bufs=8, space="PSUM") as gp,
        ):
            TC = 64
            outsT_v = [o[:].rearrange("p (b t) -> p b t", t=SEQ) for o in outsT]
            xpb = None
            for t in range(SEQ):
                if t % TC == 0:
                    xpb = xpp.tile([128, MT * BL * TC], bf16, tag="xpb")
                    xpb_w = xpb[:].rearrange("p (m b t) -> p m b t", b=BL, t=TC)
                    for m in range(MT):
                        nc.sync.dma_start(
                            out=xpb_w[:, m], in_=xpd[m, :, :, t : t + TC]
                        )
                xpb_v = xpb[:].rearrange("p (m b t) -> p m b t", b=BL, t=TC)
                if t == 0:
                    rhs_h = [h0_sb[:, k * BL : (k + 1) * BL] for k in range(KC)]
                else:
                    rhs_h = [outsT_v[k][:, :, t - 1] for k in range(KC)]
                gates = lp.tile([128, MT * BL], f32, tag="gates")
                for grp in range(4):
                    pg = gp.tile([128, 4 * BL], f32, tag="pg")
                    for mm in range(4):
                        m = grp * 4 + mm
                        for k in range(KC):
                            nc.tensor.matmul(
                                out=pg[:, mm * BL : (mm + 1) * BL],
                                lhsT=whh_sb[
                                    :, k * G + m * 128 : k * G + (m + 1) * 128
                                ],
                                rhs=rhs_h[k],
                                start=(k == 0),
                                stop=(k == KC - 1),
                            )
                    nc.vector.tensor_tensor(
                        out=gates[
                            :, grp * 4 * BL : (grp + 1) * 4 * BL
                        ].rearrange("p (m b) -> p m b", b=BL),
                        in0=pg[:].rearrange("p (m b) -> p m b", b=BL),
                        in1=xpb_v[:, grp * 4 : (grp + 1) * 4, :, t % TC],
                        op=OP.add,
                    )
                acts = lp.tile([128, MT * BL], f32, tag="acts")
                s_if = slice(0, 8 * BL)
                s_g = slice(8 * BL, 12 * BL)
                s_o = slice(12 * BL, 16 * BL)
                nc.scalar.activation(acts[:, s_if], gates[:, s_if], AF.Sigmoid)
                nc.scalar.activation(acts[:, s_g], gates[:, s_g], AF.Tanh)
                nc.scalar.activation(acts[:, s_o], gates[:, s_o], AF.Sigmoid)
                t1 = lp.tile([128, KC * BL], f32, tag="t1")
                t2 = lp.tile([128, KC * BL], f32, tag="t2")
                nc.vector.tensor_tensor(
                    out=t1[:], in0=acts[:, 4 * BL : 8 * BL], in1=c_sb[:], op=OP.mult
                )
                nc.vector.tensor_tensor(
                    out=t2[:], in0=acts[:, 0 : 4 * BL], in1=acts[:, s_g], op=OP.mult
                )
                nc.vector.tensor_tensor(out=c_sb[:], in0=t1[:], in1=t2[:], op=OP.add)
                tnc = lp.tile([128, KC * BL], f32, tag="tnc")
                nc.scalar.activation(tnc[:], c_sb[:], AF.Tanh)
                nc.vector.tensor_tensor(
                    out=h_sb[:], in0=acts[:, s_o], in1=tnc[:], op=OP.mult
                )
                for k in range(KC):
                    nc.vector.tensor_copy(
                        out=outsT_v[k][:, :, t], in_=h_sb[:, k * BL : (k + 1) * BL]
                    )

        # ---------- phase C: attention (own 8 batches) ----------
        with (
            tc.tile_pool(name="ap", bufs=2) as ap,
            tc.tile_pool(name="app", bufs=2, space="PSUM") as app,
            tc.tile_pool(name="wbp", bufs=2, space="PSUM") as wbp,
        ):
            # v = attn_w.T @ q ; q = h_sb column of replicated batch 63
            pv = app.tile([128, KC], f32, tag="pv")
            for hc in range(KC):
                for k in range(KC):
                    nc.tensor.matmul(
                        out=pv[:, hc : hc + 1],
                        lhsT=attnw_sb[:, k * H + hc * 128 : k * H + (hc + 1) * 128],
                        rhs=h_sb[:, k * BL + 8 : k * BL + 9],
                        start=(k == 0),
                        stop=(k == KC - 1),
                    )
            nc.vector.tensor_copy(out=vbf_sb[:], in_=pv[:])

            nc.gpsimd.memset(scores_sb[:], -1.0e30)
            for b in range(8):
                for th in range(2):
                    ncols = 512 if th == 0 else 511
                    ps = app.tile([128, 512], f32, tag="ps")
                    for hc in range(KC):
                        nc.tensor.matmul(
                            out=ps[:1, :ncols],
                            lhsT=vbf_sb[:, hc : hc + 1],
                            rhs=outsT[hc][
                                :, b * SEQ + th * 512 : b * SEQ + th * 512 + ncols
                            ],
                            start=(hc == 0),
                            stop=(hc == KC - 1),
                        )
                    srow = ap.tile([1, 512], f32, tag="srow")
                    nc.vector.tensor_copy(out=srow[:1, :ncols], in_=ps[:1, :ncols])
                    nc.sync.dma_start(
                        out=scores_sb[b : b + 1, th * 512 : th * 512 + ncols],
                        in_=srow[:1, :ncols],
                    )
            mx = ap.tile([8, 1], f32, tag="mx")
            nc.vector.tensor_reduce(
                out=mx[:], in_=scores_sb[:], axis=AX.X, op=OP.max
            )
            nmx = ap.tile([8, 1], f32, tag="nmx")
            nc.vector.tensor_scalar_mul(out=nmx[:], in0=mx[:], scalar1=-1.0)
            ssum = ap.tile([8, 1], f32, tag="ssum")
            nc.scalar.activation(
                wts_sb[:], scores_sb[:], AF.Exp, bias=nmx[:, 0:1], accum_out=ssum[:]
            )
            rs = ap.tile([8, 1], f32, tag="rs")
            nc.vector.reciprocal(out=rs[:], in_=ssum[:])
            nc.vector.tensor_scalar_mul(out=wtsn_sb[:], in0=wts_sb[:], scalar1=rs[:, 0:1])

            for b in range(8):
                wr0 = ap.tile([1, 1024], bf16, tag="wr0")
                nc.sync.dma_start(out=wr0[:], in_=wtsn_sb[b : b + 1, :])
                wbc = wbp.tile([128, 1024], f32, tag="wbc")
                for th in range(2):
                    nc.tensor.matmul(
                        out=wbc[:, th * 512 : (th + 1) * 512],
                        lhsT=ones_sb[:1, :],
                        rhs=wr0[:1, th * 512 : (th + 1) * 512],
                        start=True,
                        stop=True,
                    )
                for hc in range(KC):
                    prod = ap.tile([128, 1024], f32, tag="prod")
                    nc.vector.tensor_tensor(
                        out=prod[:],
                        in0=outsT[hc][:, b * SEQ : (b + 1) * SEQ],
                        in1=wbc[:],
                        op=OP.mult,
                    )
                    nc.vector.tensor_reduce(
                        out=cntxT_sb[:, hc * 8 + b : hc * 8 + b + 1],
                        in_=prod[:],
                        axis=AX.X,
                        op=OP.add,
                    )

        # ---------- phase D: final layer + log_softmax ----------
        with (
            tc.tile_pool(name="fp", bufs=6) as fp,
            tc.tile_pool(name="fpp", bufs=4, space="PSUM") as fpp,
        ):
            nc.vector.tensor_copy(out=catT_sb[:, 0:32], in_=cntxT_sb[:])
            nc.vector.tensor_copy(
                out=catT_sb[:, 32:64].rearrange("p (k b) -> p k b", b=8),
                in_=h_sb[:].rearrange("p (k b) -> p k b", b=BL)[:, :, 0:8],
            )
            mxc = fp.tile([8, HOW_NT], f32, tag="mxc")
            for n in range(HOW_NT):
                pl = fpp.tile([128, NV], f32, tag="pl")
                for c in range(8):
                    wt = fp.tile([128, NV], bf16, tag="wt")
                    nc.sync.dma_start(
                        out=wt[:], in_=how_d[c, :, n * NV : (n + 1) * NV]
                    )
                    nc.tensor.matmul(
                        out=pl[:8, :],
                        lhsT=catT_sb[:, c * 8 : (c + 1) * 8],
                        rhs=wt[:],
                        start=(c == 0),
                        stop=(c == 7),
                    )
                hb = fp.tile([8, NV], f32, tag="hb")
                nc.sync.dma_start(out=hb[:], in_=hob_d[:, n * NV : (n + 1) * NV])
                lgc = fp.tile([8, NV], f32, tag="lgc")
                nc.vector.tensor_tensor(
                    out=lgc[:], in0=pl[:8, :], in1=hb[:], op=OP.add
                )
                nc.vector.tensor_reduce(
                    out=mxc[:, n : n + 1], in_=lgc[:], axis=AX.X, op=OP.max
                )
                nc.sync.dma_start(
                    out=logits_d[:, n * NV : (n + 1) * NV], in_=lgc[:]
                )
            mx2 = fp.tile([8, 1], f32, tag="mx2")
            nc.vector.tensor_reduce(out=mx2[:], in_=mxc[:], axis=AX.X, op=OP.max)
            nmx2 = fp.tile([8, 1], f32, tag="nmx2")
            nc.vector.tensor_scalar_mul(out=nmx2[:], in0=mx2[:], scalar1=-1.0)
            esum = fp.tile([8, HOW_NT], f32, tag="esum")
            for n in range(HOW_NT):
                lgr = fp.tile([8, NV], f32, tag="lgr")
                nc.sync.dma_start(
                    out=lgr[:], in_=logits_d[:, n * NV : (n + 1) * NV]
                )
                ech = fp.tile([8, NV], f32, tag="ech")
                nc.scalar.activation(
                    ech[:], lgr[:], AF.Exp,
                    bias=nmx2[:, 0:1], accum_out=esum[:, n : n + 1],
                )
            tot = fp.tile([8, 1], f32, tag="tot")
            nc.vector.tensor_reduce(out=tot[:], in_=esum[:], axis=AX.X, op=OP.add)
            lns = fp.tile([8, 1], f32, tag="lns")
            nc.scalar.activation(lns[:], tot[:], AF.Ln)
            ofs = fp.tile([8, 1], f32, tag="ofs")
            nc.vector.tensor_tensor(
                out=ofs[:], in0=nmx2[:], in1=lns[:], op=OP.subtract
            )
            for n in range(HOW_NT):
                lgr2 = fp.tile([8, NV], f32, tag="lgr2")
                nc.sync.dma_start(
                    out=lgr2[:], in_=logits_d[:, n * NV : (n + 1) * NV]
                )
                lgo = fp.tile([8, NV], f32, tag="lgo")
                nc.vector.tensor_scalar_add(
                    out=lgo[:], in0=lgr2[:], scalar1=ofs[:, 0:1]
                )
                nc.sync.dma_start(
                    out=logp_o[:, n * NV : (n + 1) * NV], in_=lgo[:]
                )
            nc.sync.dma_start(
                out=hT_o[:].rearrange("p (k b) -> p k b", b=8),
                in_=h_sb[:].rearrange("p (k b) -> p k b", b=BL)[:, :, 0:8],
            )
            nc.sync.dma_start(
                out=cT_o[:].rearrange("p (k b) -> p k b", b=8),
                in_=c_sb[:].rearrange("p (k b) -> p k b", b=BL)[:, :, 0:8],
            )

    nc.compile()
    return nc


def _prep_inputs(non_terminal_input, h0, c0, emb_table, w_ih, w_hh, b_ih, b_hh,
                 attn_w, h2o_w, h2o_b):
    tok = np.asarray(non_terminal_input)[..., 0].astype(np.int32)  # [S, B]
    emb = np.ascontiguousarray(np.asarray(emb_table, dtype=np.float32))
    wih = np.ascontiguousarray(
        np.asarray(w_ih, np.float32).T.reshape(EC, 128, G).astype(BF16)
    )
    whh = np.ascontiguousarray(
        np.asarray(w_hh, np.float32).T.reshape(KC, 128, G).astype(BF16)
    )
    bias = np.ascontiguousarray(
        (np.asarray(b_ih, np.float32) + np.asarray(b_hh, np.float32))
        .reshape(MT, 128).T.astype(np.float32)
    )
    attnw = np.ascontiguousarray(
        np.asarray(attn_w, np.float32).reshape(KC, 128, H)
    )
    how = np.ascontiguousarray(
        np.asarray(h2o_w, np.float32).T.reshape(8, 128, V).astype(BF16)
    )
    hob = np.ascontiguousarray(
        np.broadcast_to(np.asarray(h2o_b, np.float32), (8, V)).copy()
    )
    h0 = np.asarray(h0, np.float32)
    c0 = np.asarray(c0, np.float32)

    in_maps = []
    for c in range(NCORES):
        gb = [c * 8 + j for j in range(8)] + [B - 1]
        idx = np.ascontiguousarray(
            tok[:, gb].T.reshape(BL, 8, 128).reshape(BL * 8, 128, 1)
        )
        h0t = np.ascontiguousarray(
            h0[gb].T.reshape(KC, 128, BL).transpose(1, 0, 2).reshape(128, KC * BL)
        ).astype(BF16)
        c0t = np.ascontiguousarray(
            c0[gb].T.reshape(KC, 128, BL).transpose(1, 0, 2).reshape(128, KC * BL)
        )
        in_maps.append({
            "idx": idx, "emb": emb, "wih": wih, "whh": whh, "bias": bias,
            "h0t": h0t, "c0t": c0t, "attnw": attnw, "how": how, "hob": hob,
        })
    return in_maps


def kernel(**inputs):
    global LAST_RESULTS
    from concourse import bass_utils

    if "nc" not in _CACHE:
        _CACHE["nc"] = _build()
    nc = _CACHE["nc"]

    in_maps = _prep_inputs(**inputs)
    res = bass_utils.run_bass_kernel_spmd(
        nc, in_maps, list(range(NCORES)), trace=TRACE
    )
    LAST_RESULTS = res
    outs = res.results

    def unshuf(buf):
        # device layout [128 p, KC, 8 b] -> [8 b, KC*128 h]
        return np.ascontiguousarray(
            np.transpose(buf.reshape(128, KC, 8), (2, 1, 0)).reshape(8, H)
        )

    logp = np.concatenate([outs[c]["logp_o"] for c in range(NCORES)], axis=0)
    hT = np.concatenate([unshuf(outs[c]["hT_o"]) for c in range(NCORES)], axis=0)
    cT = np.concatenate([unshuf(outs[c]["cT_o"]) for c in range(NCORES)], axis=0)
    return logp.astype(np.float32), hT.astype(np.float32), cT.astype(np.float32)
